# revision 15
# baseline (speedup 1.0000x reference)
"""Trainium2 Bass kernel for nn_LocalAggBlock (KNN + gather + MLP + maxpool).

Math (exact refactoring of the reference):
  y[n,k] = relu(concat[f_n, f_nb-f_n, p_nb-p_n] @ W + b)
         = relu(a_n + gh[idx[n,k]])
  where a_n  = f_n @ (W1-W2) - p_n @ W3          (per query point)
        gh_m = f_m @ W2 + p_m @ W3 + b            (per reference point)
  out[n] = max_k y[n,k] = relu(a_n + max_k gh[idx[n,k]])   (relu/max commute,
           a_n constant over k)

  KNN ranking uses s'[n,m] = 2 p_n . p_m - ||p_m||^2 (larger = closer; the
  ||p_n||^2 term is constant per row and does not change the ranking).

Sharding: 8 cores = (batch b in 0..1) x (quarter of N).  Each core handles
2048 query points against all 8192 points of its batch.

Host<->device traffic is the bottleneck (axon-tunneled devices), so each
core only uploads its own 2048-point slice (feat in bf16 + coords/W in
f32); full-batch tables are rebuilt on device with NeuronLink AllGathers:
  - coords AllGather  -> transposed ref coords for the distance matmuls
  - gh AllGather      -> full 8192-row gather table (computed locally first)
  - output AllGather  -> full output on every core, so the host fetches a
    single shard (one D2H round trip instead of eight)
"""

import numpy as np
import ml_dtypes

import concourse.bacc as bacc
import concourse.bass as bass
import concourse.mybir as mybir
import concourse.tile as tile
from concourse.bass import IndirectOffsetOnAxis
from concourse.masks import make_identity

F32 = mybir.dt.float32
BF16 = mybir.dt.bfloat16
U32 = mybir.dt.uint32
AF = mybir.ActivationFunctionType
NEG = -3.0e38

B, N, C = 2, 8192, 64
KNN = 16
NCORES = 8
QPC = B * N // NCORES          # queries per core (2048)
CT_ROWS = 96                   # coords^T (3, 2048) packed as (96, 64)
CB_ROWS = CT_ROWS + 131 + 1    # + W (131, 64) + b (1, 64)
GROUPS4 = [[0, 1, 2, 3], [4, 5, 6, 7]]   # per-batch replica groups
GROUPS8 = [list(range(NCORES))]


def build_kernel(n_refs=N, n_q=QPC):
    n_chunk = n_refs // 512    # ref chunks per query block
    n_qblk = n_q // 128        # query blocks
    n_grp = NCORES // B        # cores per batch

    nc = bacc.Bacc(None, target_bir_lowering=False)
    fblob = nc.dram_tensor("fblob", [n_q, C], BF16, kind="ExternalInput")
    cblob = nc.dram_tensor("cblob", [CB_ROWS, 64], F32, kind="ExternalInput")
    out_d = nc.dram_tensor("out", [NCORES * n_q, C], BF16, kind="ExternalOutput")

    ct_loc = nc.dram_tensor("ct_loc", [CT_ROWS, 64], F32, kind="Internal")
    ct_all = nc.dram_tensor("ct_all", [n_grp * CT_ROWS, 64], F32, kind="Internal")
    gh_loc = nc.dram_tensor("gh_loc", [n_q, C], F32, kind="Internal")
    gh_d = nc.dram_tensor("gh", [n_refs, C], F32, kind="Internal")
    out_loc = nc.dram_tensor("out_loc", [n_q, C], BF16, kind="Internal")
    out_bounce = nc.dram_tensor("out_bounce", [NCORES * n_q, C], BF16,
                                kind="Internal")

    with tile.TileContext(nc) as tc:
        with tc.tile_pool(name="persist", bufs=1) as pp:
            ident = pp.tile([128, 128], F32)
            make_identity(nc, ident[:])

            # --- coords AllGather (issued first; refsT consumes it later) ---
            nc.sync.dma_start(ct_loc[:], cblob[0:CT_ROWS, :])
            nc.gpsimd.collective_compute(
                "AllGather", mybir.AluOpType.bypass, replica_groups=GROUPS4,
                ins=[ct_loc[:]], outs=[ct_all[:]])

            # --- weights ---
            wa = pp.tile([C, C], F32)
            wb = pp.tile([C, C], F32)
            wd = pp.tile([C, C], F32)     # W1 - W2
            wc = pp.tile([3, C], F32)
            negwc = pp.tile([3, C], F32)
            bsb = pp.tile([1, C], F32)
            ones1 = pp.tile([1, 128], F32)
            neg3 = pp.tile([3, 1], F32)
            nc.sync.dma_start(wa[:], cblob[CT_ROWS:CT_ROWS + 64, :])
            nc.sync.dma_start(wb[:], cblob[CT_ROWS + 64:CT_ROWS + 128, :])
            nc.sync.dma_start(wc[:], cblob[CT_ROWS + 128:CT_ROWS + 131, :])
            nc.sync.dma_start(bsb[:], cblob[CT_ROWS + 131:CT_ROWS + 132, :])
            nc.vector.tensor_sub(wd[:], wa[:], wb[:])
            nc.vector.tensor_scalar_mul(negwc[:], wc[:], -1.0)
            nc.vector.memset(ones1[:], 1.0)
            nc.vector.memset(neg3[:], -1.0)

            # --- local query coords (transposed) ---
            qTraw = pp.tile([3, n_q], F32)      # raw query coords^T
            qT = pp.tile([4, n_q], F32)         # rows 0-2: 2*p_q^T, row 3: ones
            nc.sync.dma_start(
                qTraw[:], cblob[0:CT_ROWS, :].rearrange("(c r) f -> c (r f)", c=3))
            nc.vector.memset(qT[:], 1.0)  # row 3 stays 1.0
            nc.vector.tensor_scalar_mul(qT[0:3, :], qTraw[:], 2.0)

            # --- full-batch ref coords (transposed) from the AllGather ---
            refsT = pp.tile([4, n_refs], F32)   # rows 0-2: p^T, row 3: -||p||^2
            for g in range(n_grp):
                nc.sync.dma_start(
                    refsT[0:3, g * n_q:(g + 1) * n_q],
                    ct_all[g * CT_ROWS:(g + 1) * CT_ROWS, :].rearrange(
                        "(c r) f -> c (r f)", c=3))

            sq = pp.tile([3, n_refs], F32)
            nc.vector.tensor_mul(sq[:], refsT[0:3, :], refsT[0:3, :])

            a_all = pp.tile([128, n_qblk * C], F32)
            normrow = pp.tile([1, n_refs], F32)

            with tc.tile_pool(name="setup_psum", bufs=2, space="PSUM") as sp, \
                 tc.tile_pool(name="setup_sb", bufs=3) as sb:
                # row 3 of refsT: -(x^2+y^2+z^2) via PE partition-reduce
                for ch in range(n_chunk):
                    psum_n = sp.tile([1, 512], F32, tag="n")
                    nc.tensor.matmul(psum_n[:], neg3[:], sq[:, ch * 512:(ch + 1) * 512],
                                     start=True, stop=True)
                    nc.scalar.activation(normrow[0:1, ch * 512:(ch + 1) * 512],
                                         psum_n[:], AF.Copy)
                # compute engines can't start at partition 3; DMA can
                nc.sync.dma_start(refsT[3:4, :], normrow[:])

                # gh_loc[m] = f_m @ W2 + p_m @ W3 + b  and
                # a[n]      = f_n @ (W1-W2) - p_n @ W3   (same rows; share f^T)
                for rb in range(n_qblk):
                    r0 = rb * 128
                    fblk = sb.tile([128, C], BF16, tag="fblk")
                    nc.sync.dma_start(fblk[:], fblob[r0:r0 + 128, :])
                    f32blk = sb.tile([128, C], F32, tag="f32blk")
                    nc.scalar.activation(f32blk[:], fblk[:], AF.Copy)
                    psum_t = sp.tile([C, 128], F32, tag="t")
                    nc.tensor.transpose(psum_t[:], f32blk[:], ident[:])
                    ftT = sb.tile([C, 128], F32, tag="ftT")
                    nc.scalar.activation(ftT[:], psum_t[:], AF.Copy)

                    psum_g = sp.tile([128, C], F32, tag="g")
                    nc.tensor.matmul(psum_g[:], ftT[:], wb[:], start=True, stop=False)
                    nc.tensor.matmul(psum_g[:], qTraw[:, r0:r0 + 128], wc[:],
                                     start=False, stop=False)
                    nc.tensor.matmul(psum_g[:], ones1[:], bsb[:], start=False, stop=True)
                    ghblk = sb.tile([128, C], F32, tag="ghblk")
                    nc.scalar.activation(ghblk[:], psum_g[:], AF.Copy)
                    nc.sync.dma_start(gh_loc[r0:r0 + 128, :], ghblk[:])

                    psum_a = sp.tile([128, C], F32, tag="a")
                    nc.tensor.matmul(psum_a[:], ftT[:], wd[:], start=True, stop=False)
                    nc.tensor.matmul(psum_a[:], qTraw[:, r0:r0 + 128], negwc[:],
                                     start=False, stop=True)
                    nc.scalar.activation(a_all[:, rb * C:(rb + 1) * C], psum_a[:],
                                         AF.Copy)

            # full 8192-row gather table
            nc.gpsimd.collective_compute(
                "AllGather", mybir.AluOpType.bypass, replica_groups=GROUPS4,
                ins=[gh_loc[:]], outs=[gh_d[:]])

            # --- main loop: per 128-query block ---
            with tc.tile_pool(name="mm_psum", bufs=6, space="PSUM") as mp, \
                 tc.tile_pool(name="srow", bufs=2) as spool, \
                 tc.tile_pool(name="small", bufs=4) as smp:
                for qb in range(n_qblk):
                    q0 = qb * 128
                    S = spool.tile([128, n_refs], F32, tag="S")
                    for ch in range(n_chunk):
                        c0 = ch * 512
                        psum_s = mp.tile([128, 512], F32, tag="s")
                        nc.tensor.matmul(psum_s[:], qT[:, q0:q0 + 128],
                                         refsT[:, c0:c0 + 512], start=True, stop=True)
                        nc.scalar.activation(S[:, c0:c0 + 512], psum_s[:], AF.Copy)

                    v = smp.tile([128, 16], F32, tag="v")
                    idx = smp.tile([128, 16], U32, tag="idx")
                    nc.vector.max(v[:, 0:8], S[:])
                    nc.vector.max_index(idx[:, 0:8], v[:, 0:8], S[:])
                    nc.vector.match_replace(S[:], v[:, 0:8], S[:], NEG)
                    nc.vector.max(v[:, 8:16], S[:])
                    nc.vector.max_index(idx[:, 8:16], v[:, 8:16], S[:])

                    nb = smp.tile([128, KNN * C], F32, tag="nb")
                    # HW indirect DMA consumes one offset per partition, so
                    # gather one 64-wide slab per neighbor k.
                    for k in range(KNN):
                        nc.gpsimd.indirect_dma_start(
                            out=nb[:, k * C:(k + 1) * C], out_offset=None,
                            in_=gh_d[:],
                            in_offset=IndirectOffsetOnAxis(ap=idx[:, k:k + 1], axis=0))

                    mx = smp.tile([128, C], F32, tag="mx")
                    nc.vector.tensor_reduce(
                        mx[:], nb[:].rearrange("p (k c) -> p c k", k=KNN),
                        axis=mybir.AxisListType.X, op=mybir.AluOpType.max)
                    nc.vector.tensor_add(mx[:], mx[:], a_all[:, qb * C:(qb + 1) * C])
                    ob = smp.tile([128, C], BF16, tag="ob")
                    nc.scalar.activation(ob[:], mx[:], AF.Relu)
                    nc.sync.dma_start(out_loc[q0:q0 + 128, :], ob[:])

            # gather the full output on every core; host reads one shard
            nc.gpsimd.collective_compute(
                "AllGather", mybir.AluOpType.bypass, replica_groups=GROUPS8,
                ins=[out_loc[:]], outs=[out_bounce[:]])
            nc.sync.dma_start(out_d[:], out_bounce[:])

    return nc


def _f32_to_bf16(a):
    """Truncating f32 -> bf16 (adds <=2^-8 rel error; gate is 2e-2, measured
    output impact 4.2e-3 vs 3.0e-3 for round-to-nearest, and it packs ~6 ms
    faster per call than RNE)."""
    u = np.ascontiguousarray(a, np.float32).view(np.uint32)
    return (u >> 16).astype(np.uint16).view(ml_dtypes.bfloat16)


def _bf16_to_f32(a):
    u = np.empty(a.shape, np.uint32)
    u[:] = np.ascontiguousarray(a).view(np.uint16)
    u <<= np.uint32(16)
    return u.view(np.float32)


def _pack_fblob(feat):
    feat = np.ascontiguousarray(feat, np.float32).reshape(NCORES * QPC, C)
    return _f32_to_bf16(feat)                       # [8*2048, 64] bf16


def _pack_cblob(coords_knn, W, b):
    """Per-core coord/weight blobs, concatenated core-major."""
    coords_knn = coords_knn.astype(np.float32, copy=False)
    cb = np.empty((NCORES * CB_ROWS, 64), np.float32)
    n_grp = NCORES // B
    for core in range(NCORES):
        bb = core // n_grp
        q0 = (core % n_grp) * QPC
        off = core * CB_ROWS
        cb[off:off + CT_ROWS].reshape(3, QPC)[:] = coords_knn[bb, q0:q0 + QPC].T
        cb[off + CT_ROWS:off + CT_ROWS + 131] = W
        cb[off + CT_ROWS + 131] = b
    return cb                                       # [8*228, 64] f32


_CACHE = {}


def _get_runner():
    if "runner" in _CACHE:
        return _CACHE["runner"]

    import jax
    from jax.sharding import Mesh, PartitionSpec
    from jax.experimental.shard_map import shard_map
    from concourse import bass2jax

    nc = build_kernel()
    nc.compile()
    bass2jax.install_neuronx_cc_hook()

    partition_name = nc.partition_id_tensor.name if nc.partition_id_tensor else None
    in_names, out_names, out_avals = [], [], []
    for alloc in nc.m.functions[0].allocations:
        if not isinstance(alloc, mybir.MemoryLocationSet):
            continue
        name = alloc.memorylocations[0].name
        if alloc.kind == "ExternalInput":
            if name != partition_name:
                in_names.append(name)
        elif alloc.kind == "ExternalOutput":
            out_names.append(name)
            out_avals.append(jax.core.ShapedArray(
                tuple(alloc.tensor_shape), mybir.dt.np(alloc.dtype)))
    assert in_names == ["fblob", "cblob"] and out_names == ["out"], \
        (in_names, out_names)
    n_params, n_outs = len(in_names), len(out_names)
    in_names_full = in_names + out_names + ([partition_name] if partition_name else [])
    donate = tuple(range(n_params, n_params + n_outs))

    def _body(*args):
        operands = list(args)
        if partition_name:
            operands.append(bass2jax.partition_id_tensor())
        return tuple(bass2jax._bass_exec_p.bind(
            *operands, out_avals=tuple(out_avals), in_names=tuple(in_names_full),
            out_names=tuple(out_names), lowering_input_output_aliases=(),
            sim_require_finite=True, sim_require_nnan=True, nc=nc))

    devices = jax.devices()[:NCORES]
    mesh = Mesh(np.asarray(devices), ("core",))
    sharded = jax.jit(
        shard_map(_body, mesh=mesh,
                  in_specs=(PartitionSpec("core"),) * (n_params + n_outs),
                  out_specs=(PartitionSpec("core"),) * n_outs, check_rep=False),
        donate_argnums=donate, keep_unused=True)
    shardspec = jax.sharding.NamedSharding(mesh, PartitionSpec("core"))
    _CACHE["runner"] = (sharded, out_avals, shardspec)

    # Warm every dispatch path (incl. donated-Array args) so the caller's
    # first timed calls run the steady-state fast path.
    fz = jax.device_put(
        np.zeros((NCORES * QPC, C), ml_dtypes.bfloat16), shardspec)
    cz = np.zeros((NCORES * CB_ROWS, 64), np.float32)
    oz = np.zeros((NCORES * out_avals[0].shape[0], out_avals[0].shape[1]),
                  out_avals[0].dtype)
    (og,) = sharded(fz, cz, oz)
    np.asarray(min(og.addressable_shards,
                   key=lambda s: s.index[0].start or 0).data)
    (og,) = sharded(fz, cz, og)
    _CACHE["prev_out"] = og
    return _CACHE["runner"]


def kernel(coords_knn, feat, W, b):
    import jax

    sharded, out_avals, shardspec = _get_runner()
    # start both uploads asynchronously so they stream during dispatch
    dfblob = jax.device_put(_pack_fblob(feat), shardspec)
    dcblob = jax.device_put(_pack_cblob(coords_knn, W, b), shardspec)

    out_buf = _CACHE.pop("prev_out", None)
    if out_buf is None:
        shape = out_avals[0].shape
        out_buf = np.zeros((NCORES * shape[0], shape[1]), out_avals[0].dtype)

    (out_g,) = sharded(dfblob, dcblob, out_buf)
    _CACHE["prev_out"] = out_g  # donate into the next call (fully overwritten)

    shard0 = next(s.data for s in out_g.addressable_shards
                  if (s.index[0].start or 0) == 0)
    try:
        shard0.copy_to_host_async()  # queue the D2H behind the running exec
    except Exception:
        pass
    raw = np.asarray(shard0)                        # [16384, 64] bf16
    return _bf16_to_f32(raw).reshape(B, N, C)
